# revision 17
# baseline (speedup 1.0000x reference)
"""TRN2 Bass kernel for nn_CRLoss: semi-hard-negative-mining triplet CR loss.

Strategy (data-parallel over 8 NeuronCores, no collectives):
  - Host: row-normalize img/txt/txt_cr (fp32, matches reference), build
    transposed [D, B] copies, slice per-core row blocks, labels/margin as f32.
  - Each core computes 4 row-direction similarity slabs of shape [B/8, B]:
        img_loc @ txtT   (dir_loss(sim) rows)
        txt_loc @ imgT   (dir_loss(sim.T) rows)
        img_loc @ txcT   (dir_loss(sim_cr) rows)
        txc_loc @ imgT   (dir_loss(sim_cr.T) rows)
    streamed through PSUM in [128, 512] chunks; never materialized in DRAM.
  - Mining per row: first j with (labels differ) and sim in (diag-margin, diag)
    == |S - c| < h with c = diag - margin/2, h = margin/2.  Computed as
    w = (|S-c| < h) * REVIOTA*NEQ, row-max(w) -> rv, j* = B - rv.
  - Value: gather normalized counterpart rows by j*, fp32 row-dot, then
    relu(margin - diag + dot), masked by has_valid (& margin>=0.16 if auto).
  - Cores return [128, 2] partials (base, cr); host reduces + cr_beta combine.
"""
import numpy as np

import concourse.bass as bass
import concourse.bacc as bacc
import concourse.tile as tile
from concourse import mybir
from concourse.bass_utils import run_bass_kernel_spmd

f32 = mybir.dt.float32
f32r = mybir.dt.float32r
f16 = mybir.dt.float16
i32 = mybir.dt.int32
Alu = mybir.AluOpType
Act = mybir.ActivationFunctionType
AX = mybir.AxisListType

B = 8192          # total rows
D = 512           # embedding dim
NCORES = 8
L = B // NCORES   # rows per core (1024)
MT = L // 128     # m-tiles per core (8)
KT = D // 128     # contraction tiles (4)
NG = 8            # column groups per row of the slab
GW = B // NG      # group width (1024)
CH = GW // 512    # 512-wide matmul chunks per group (2)

_CACHE = {}


def _build(auto_flag, mm_dtype):
    """Build the SPMD Bass graph (one program shared by all 8 cores)."""
    nc = bacc.Bacc(None, target_bir_lowering=False, debug=True)

    # full matrices (shared np arrays across cores)
    aT_d = nc.declare_dram_parameter("aT", [D, B], mm_dtype, isOutput=False)
    bT_d = nc.declare_dram_parameter("bT", [D, B], mm_dtype, isOutput=False)
    cT_d = nc.declare_dram_parameter("cT", [D, B], mm_dtype, isOutput=False)
    an_d = nc.declare_dram_parameter("an", [B, D], f32, isOutput=False)
    bn_d = nc.declare_dram_parameter("bn", [B, D], f32, isOutput=False)
    cn_d = nc.declare_dram_parameter("cn", [B, D], f32, isOutput=False)
    labrow_d = nc.declare_dram_parameter("labrow", [128, B], f32, isOutput=False)
    riota_d = nc.declare_dram_parameter("riota", [128, B], f32, isOutput=False)
    # per-core slices
    laT_d = nc.declare_dram_parameter("laT", [D, L], mm_dtype, isOutput=False)
    lbT_d = nc.declare_dram_parameter("lbT", [D, L], mm_dtype, isOutput=False)
    lcT_d = nc.declare_dram_parameter("lcT", [D, L], mm_dtype, isOutput=False)
    lan_d = nc.declare_dram_parameter("lan", [L, D], f32, isOutput=False)
    lbn_d = nc.declare_dram_parameter("lbn", [L, D], f32, isOutput=False)
    lcn_d = nc.declare_dram_parameter("lcn", [L, D], f32, isOutput=False)
    lab_d = nc.declare_dram_parameter("lab", [L, 1], f32, isOutput=False)
    marg_d = nc.declare_dram_parameter("marg", [L, 1], f32, isOutput=False)
    out_d = nc.declare_dram_parameter("out", [128, 2], f32, isOutput=True)

    with tile.TileContext(nc) as tc:
        with (
            tc.tile_pool(name="lhs", bufs=1) as lhs_p,
            tc.tile_pool(name="rhs", bufs=2) as rhs_p,
            tc.tile_pool(name="bc", bufs=1) as bc_p,
            tc.tile_pool(name="rr", bufs=2) as rr_p,
            tc.tile_pool(name="aw", bufs=2) as aw_p,
            tc.tile_pool(name="sm", bufs=1) as sm_p,
            tc.tile_pool(name="post", bufs=1) as post_p,
            tc.tile_pool(name="ps", bufs=8, space="PSUM") as ps_p,
        ):
            # ---------------- prework: local loads, diag dots, margins ----
            lab_t = sm_p.tile([128, MT], f32, tag="lab")
            nc.sync.dma_start(out=lab_t, in_=lab_d.rearrange("(m p) o -> p m o", p=128))
            marg_t = sm_p.tile([128, MT], f32, tag="marg")
            nc.sync.dma_start(out=marg_t, in_=marg_d.rearrange("(m p) o -> p m o", p=128))

            sm_t = sm_p.tile([128, MT], f32, tag="smv")       # diag(sim)
            smcr_t = sm_p.tile([128, MT], f32, tag="smcr")    # diag(sim_cr)
            scr1 = sm_p.tile([128, D], f32, tag="scr1")
            scr2 = sm_p.tile([128, D], f32, tag="scr2")
            for m in range(MT):
                r0 = m * 128
                la_m = post_p.tile([128, D], f32, tag="aloc")
                nc.sync.dma_start(out=la_m, in_=lan_d[r0:r0 + 128, :])
                lb_m = post_p.tile([128, D], f32, tag="bloc")
                nc.sync.dma_start(out=lb_m, in_=lbn_d[r0:r0 + 128, :])
                lc_m = post_p.tile([128, D], f32, tag="cloc")
                nc.sync.dma_start(out=lc_m, in_=lcn_d[r0:r0 + 128, :])
                nc.vector.scalar_tensor_tensor(
                    out=scr1[:], in0=la_m[:], scalar=1.0, in1=lb_m[:],
                    op0=Alu.mult, op1=Alu.mult, accum_out=sm_t[:, m:m + 1])
                nc.vector.scalar_tensor_tensor(
                    out=scr2[:], in0=la_m[:], scalar=1.0, in1=lc_m[:],
                    op0=Alu.mult, op1=Alu.mult, accum_out=smcr_t[:, m:m + 1])

            # margin_cr
            margcr_t = sm_p.tile([128, MT], f32, tag="margcr")
            if auto_flag:
                asm = sm_p.tile([128, MT], f32, tag="asm")
                asmcr = sm_p.tile([128, MT], f32, tag="asmcr")
                lam = sm_p.tile([128, MT], f32, tag="lam")
                nc.scalar.activation(out=asm[:], in_=sm_t[:], func=Act.Abs)
                nc.scalar.activation(out=asmcr[:], in_=smcr_t[:], func=Act.Abs)
                nc.vector.reciprocal(out=asm[:], in_=asm[:])
                nc.vector.tensor_tensor(out=lam[:], in0=asmcr[:], in1=asm[:], op=Alu.mult)
                nc.vector.tensor_scalar(out=lam[:], in0=lam[:], scalar1=1.0, scalar2=1.0,
                                        op0=Alu.min, op1=Alu.add)       # lam+1 in [1,2]
                nc.vector.tensor_tensor(out=margcr_t[:], in0=lam[:], in1=marg_t[:], op=Alu.mult)
                nc.vector.tensor_scalar(out=margcr_t[:], in0=margcr_t[:], scalar1=0.5, scalar2=None, op0=Alu.mult)
            else:
                nc.vector.tensor_scalar(out=margcr_t[:], in0=marg_t[:], scalar1=0.5, scalar2=None, op0=Alu.mult)

            # per slab-class constants: negc = margin/2 - diag (ACT bias), h = margin/2,
            # bval = margin - diag, ok mask
            negc_b = sm_p.tile([128, MT], f32, tag="negc_b")
            negc_c = sm_p.tile([128, MT], f32, tag="negc_c")
            h_b = sm_p.tile([128, MT], f32, tag="h_b")
            h_c = sm_p.tile([128, MT], f32, tag="h_c")
            bv_b = sm_p.tile([128, MT], f32, tag="bv_b")
            bv_c = sm_p.tile([128, MT], f32, tag="bv_c")
            ok_b = sm_p.tile([128, MT], f32, tag="ok_b")
            ok_c = sm_p.tile([128, MT], f32, tag="ok_c")
            nc.vector.tensor_scalar(out=h_b[:], in0=marg_t[:], scalar1=0.5, scalar2=None, op0=Alu.mult)
            nc.vector.tensor_scalar(out=h_c[:], in0=margcr_t[:], scalar1=0.5, scalar2=None, op0=Alu.mult)
            nc.vector.tensor_tensor(out=negc_b[:], in0=h_b[:], in1=sm_t[:], op=Alu.subtract)
            nc.vector.tensor_tensor(out=negc_c[:], in0=h_c[:], in1=smcr_t[:], op=Alu.subtract)
            nc.vector.tensor_tensor(out=bv_b[:], in0=marg_t[:], in1=sm_t[:], op=Alu.subtract)
            nc.vector.tensor_tensor(out=bv_c[:], in0=margcr_t[:], in1=smcr_t[:], op=Alu.subtract)
            if auto_flag:
                nc.vector.tensor_scalar(out=ok_b[:], in0=marg_t[:], scalar1=0.16, scalar2=None, op0=Alu.is_ge)
                nc.vector.tensor_scalar(out=ok_c[:], in0=margcr_t[:], scalar1=0.16, scalar2=None, op0=Alu.is_ge)
            else:
                nc.vector.memset(ok_b[:], 1.0)
                nc.vector.memset(ok_c[:], 1.0)

            # lhsT tiles [128, KT, L]
            laT_t = lhs_p.tile([128, KT, L], mm_dtype, tag="laT")
            nc.sync.dma_start(out=laT_t, in_=laT_d.rearrange("(k p) n -> p k n", p=128))
            lbT_t = lhs_p.tile([128, KT, L], mm_dtype, tag="lbT")
            nc.sync.dma_start(out=lbT_t, in_=lbT_d.rearrange("(k p) n -> p k n", p=128))
            lcT_t = lhs_p.tile([128, KT, L], mm_dtype, tag="lcT")
            nc.sync.dma_start(out=lcT_t, in_=lcT_d.rearrange("(k p) n -> p k n", p=128))

            # slabs: (lhsT, rhs_dram, negc, h, class) ; class 0 = base, 1 = cr
            slabs = [
                (laT_t, bT_d, negc_b, h_b, 0),
                (lbT_t, aT_d, negc_b, h_b, 0),
                (laT_t, cT_d, negc_c, h_c, 1),
                (lcT_t, aT_d, negc_c, h_c, 1),
            ]

            # stats[s][m] columns per group: one tile [128, 4*MT, NG]
            stats_t = sm_p.tile([128, 4 * MT, NG], f32, tag="stats")

            # ---------------- main loop --------------------------------
            for g in range(NG):
                j0 = g * GW
                # rhs tiles for the 3 distinct matrices
                rT_b = rhs_p.tile([128, KT, GW], mm_dtype, tag="rT_b")
                nc.sync.dma_start(out=rT_b, in_=bT_d[:, j0:j0 + GW].rearrange("(k p) n -> p k n", p=128))
                rT_a = rhs_p.tile([128, KT, GW], mm_dtype, tag="rT_a")
                nc.sync.dma_start(out=rT_a, in_=aT_d[:, j0:j0 + GW].rearrange("(k p) n -> p k n", p=128))
                rT_c = rhs_p.tile([128, KT, GW], mm_dtype, tag="rT_c")
                nc.sync.dma_start(out=rT_c, in_=cT_d[:, j0:j0 + GW].rearrange("(k p) n -> p k n", p=128))
                rhs_for = {id(bT_d): rT_b, id(aT_d): rT_a, id(cT_d): rT_c}

                # broadcast label row + reversed iota for this group
                labB = bc_p.tile([128, GW], f32, tag="labB")
                nc.sync.dma_start(out=labB, in_=labrow_d[:, j0:j0 + GW])
                rioB = bc_p.tile([128, GW], f32, tag="rioB")
                nc.sync.dma_start(out=rioB, in_=riota_d[:, j0:j0 + GW])

                for m in range(MT):
                    # R = (labB != lab_m) * rioB   (gpsimd to keep DVE free)
                    R_t = rr_p.tile([128, GW], f32, tag="R")
                    nc.vector.scalar_tensor_tensor(
                        out=R_t[:], in0=labB[:], scalar=lab_t[:, m:m + 1], in1=rioB[:],
                        op0=Alu.not_equal, op1=Alu.mult)

                    for s, (lhsT_t, rhs_d, negc_t, hh_t, _cls) in enumerate(slabs):
                        rT = rhs_for[id(rhs_d)]
                        a_t = aw_p.tile([128, GW], f32, tag="a")
                        for ch in range(CH):
                            c0 = ch * 512
                            psum = ps_p.tile([128, 512], f32, tag="ps")
                            for k in range(KT):
                                nc.tensor.matmul(
                                    psum[:],
                                    lhsT_t[:, k, m * 128:(m + 1) * 128],
                                    rT[:, k, c0:c0 + 512],
                                    start=(k == 0), stop=(k == KT - 1))
                            # a = |S - c| from PSUM
                            nc.scalar.activation(
                                out=a_t[:, c0:c0 + 512], in_=psum[:], func=Act.Abs,
                                bias=negc_t[:, m:m + 1], scale=1.0)
                        # w = (a < h) * R ; rowmax -> stats
                        w_t = aw_p.tile([128, GW], f32, tag="w")
                        nc.vector.scalar_tensor_tensor(
                            out=w_t[:], in0=a_t[:], scalar=hh_t[:, m:m + 1], in1=R_t[:],
                            op0=Alu.is_lt, op1=Alu.mult)
                        nc.vector.tensor_reduce(
                            out=stats_t[:, s * MT + m, g:g + 1], in_=w_t[:],
                            axis=AX.X, op=Alu.max)

            # ---------------- post: select, gather, redot, accumulate ----
            acc_t = sm_p.tile([128, 2], f32, tag="acc")
            nc.vector.memset(acc_t[:], 0.0)
            gtab = {0: bn_d, 1: an_d, 2: cn_d, 3: an_d}
            ldram = {0: lan_d, 1: lbn_d, 2: lan_d, 3: lcn_d}
            ltag = {0: "aloc", 1: "bloc", 2: "aloc", 3: "cloc"}
            bval = {0: bv_b, 1: bv_b, 2: bv_c, 3: bv_c}
            okm = {0: ok_b, 1: ok_b, 2: ok_c, 3: ok_c}
            for s in range(4):
                for m in range(MT):
                    rv = post_p.tile([128, 1], f32, tag="rv")
                    nc.vector.tensor_reduce(out=rv[:], in_=stats_t[:, s * MT + m], axis=AX.X, op=Alu.max)
                    has = post_p.tile([128, 1], f32, tag="has")
                    nc.vector.tensor_scalar(out=has[:], in0=rv[:], scalar1=0.0, scalar2=None, op0=Alu.is_gt)
                    # j = B - max(rv,1)  (clamps no-valid rows into range)
                    jf = post_p.tile([128, 1], f32, tag="jf")
                    nc.vector.tensor_scalar(out=jf[:], in0=rv[:], scalar1=1.0, scalar2=-1.0,
                                            op0=Alu.max, op1=Alu.mult)
                    nc.vector.tensor_scalar(out=jf[:], in0=jf[:], scalar1=float(B), scalar2=None, op0=Alu.add)
                    ji = post_p.tile([128, 1], i32, tag="ji")
                    nc.vector.tensor_copy(out=ji[:], in_=jf[:])
                    g_t = post_p.tile([128, D], f32, tag="g")
                    nc.gpsimd.indirect_dma_start(
                        out=g_t[:], out_offset=None, in_=gtab[s][:],
                        in_offset=bass.IndirectOffsetOnAxis(ap=ji[:, 0:1], axis=0))
                    lrow = post_p.tile([128, D], f32, tag=ltag[s])
                    nc.sync.dma_start(out=lrow, in_=ldram[s][m * 128:(m + 1) * 128, :])
                    vd = post_p.tile([128, 1], f32, tag="vd")
                    gscr = post_p.tile([128, D], f32, tag="gscr")
                    nc.vector.scalar_tensor_tensor(
                        out=gscr[:], in0=lrow[:], scalar=1.0, in1=g_t[:],
                        op0=Alu.mult, op1=Alu.mult, accum_out=vd[:, 0:1])
                    # per = relu(bval + vd) * has * ok ; acc[:, cls] += per
                    per = post_p.tile([128, 1], f32, tag="per")
                    nc.vector.tensor_tensor(out=per[:], in0=vd[:], in1=bval[s][:, m:m + 1], op=Alu.add)
                    nc.vector.tensor_scalar(out=per[:], in0=per[:], scalar1=0.0, scalar2=None, op0=Alu.max)
                    nc.vector.tensor_tensor(out=per[:], in0=per[:], in1=has[:], op=Alu.mult)
                    nc.vector.tensor_tensor(out=per[:], in0=per[:], in1=okm[s][:, m:m + 1], op=Alu.mult)
                    cls = slabs[s][4]
                    nc.vector.tensor_tensor(out=acc_t[:, cls:cls + 1], in0=acc_t[:, cls:cls + 1],
                                            in1=per[:], op=Alu.add)

            nc.sync.dma_start(out=out_d[:], in_=acc_t[:])

    nc.finalize()
    return nc


def _normalize(x):
    n = np.sqrt((x.astype(np.float32) ** 2).sum(1, keepdims=True, dtype=np.float32))
    return (x.astype(np.float32) / (n + np.float32(1e-8))).astype(np.float32)


def kernel(img, txt, txt_cr, labels, auto_margin_flag, margin, cr_beta):
    img = np.asarray(img, dtype=np.float32)
    txt = np.asarray(txt, dtype=np.float32)
    txt_cr = np.asarray(txt_cr, dtype=np.float32)
    labels_np = np.asarray(labels)
    margin_np = np.asarray(margin, dtype=np.float32).reshape(B, 1)
    auto = bool(int(auto_margin_flag))
    beta = float(np.asarray(cr_beta))

    an, bn, cn = _normalize(img), _normalize(txt), _normalize(txt_cr)
    aT = np.ascontiguousarray(an.T)
    bT = np.ascontiguousarray(bn.T)
    cT = np.ascontiguousarray(cn.T)
    labf = labels_np.astype(np.float32)
    labrow = np.ascontiguousarray(np.broadcast_to(labf.reshape(1, B), (128, B)))
    riota = np.ascontiguousarray(np.broadcast_to(
        (B - np.arange(B, dtype=np.float32)).reshape(1, B), (128, B)))

    import os
    mmdt = f32 if os.environ.get("CRL_MM_DT", "f32r") == "f32" else f32r
    key = (auto, os.environ.get("CRL_MM_DT", "f32r"))
    if key not in _CACHE:
        _CACHE[key] = _build(auto, mmdt)
    nc = _CACHE[key]

    in_maps = []
    for c in range(NCORES):
        r0, r1 = c * L, (c + 1) * L
        in_maps.append(dict(
            aT=aT, bT=bT, cT=cT, an=an, bn=bn, cn=cn,
            labrow=labrow, riota=riota,
            laT=np.ascontiguousarray(aT[:, r0:r1]),
            lbT=np.ascontiguousarray(bT[:, r0:r1]),
            lcT=np.ascontiguousarray(cT[:, r0:r1]),
            lan=an[r0:r1], lbn=bn[r0:r1], lcn=cn[r0:r1],
            lab=labf[r0:r1].reshape(L, 1),
            marg=margin_np[r0:r1],
        ))

    res = run_bass_kernel_spmd(nc, in_maps, list(range(NCORES)))
    base = np.float64(0.0)
    cr = np.float64(0.0)
    for c in range(NCORES):
        o = res.results[c]["out"]
        base += o[:, 0].sum(dtype=np.float64)
        cr += o[:, 1].sum(dtype=np.float64)
    return np.float32(base + beta * cr)
